# revision 1
# baseline (speedup 1.0000x reference)
"""Trainium2 Bass kernel for nn_Decoder_11278584119887 (self-contained).

6-layer dense transformer decoder with head-averaged attention weights.
Sharding: 8 NeuronCores = 4 batch elements x 2 sequence halves; per-layer
bf16 AllGather (pairs) exchanges the residual stream. All matmuls in bf16
with fp32 PSUM accumulation; softmax/LayerNorm path in fp32.
"""

import sys as _sys

for _p in ("/opt/trn_rl_repo",):
    if _p not in _sys.path:
        _sys.path.insert(0, _p)

"""Bass/Tile kernel for nn_Decoder: 6-layer decoder with averaged-head attention.

Sharding: 8 cores = 4 batches x 2 sequence-halves (sequence parallel).
Per core: own R=L/2 rows of one batch element. Per layer, an AllGather
(groups of 2) exchanges the bf16 residual so each core sees full-L h for
K-projection and attention values.

Layouts (per core, P=128 partitions):
  h_own   f32  [128, LT, E]   row-major residual, own rows (l = t*128+p)
  hT      bf16 chunks [e partitions] built by DMA-transpose for proj rhs
  KT      bf16 [128, ET, L]   (e' = et*128+p, m)  scores rhs
  QT      bf16 [128, ET, R]   (e', l)             scores lhsT
  zT_all  bf16 [128, MT, R]   (m, l)              sa lhsT
"""

import math
from dataclasses import dataclass, field

import numpy as np

import concourse.bass as bass
import concourse.mybir as mybir
import concourse.tile as tile

F32 = mybir.dt.float32
BF16 = mybir.dt.bfloat16
AF = mybir.ActivationFunctionType
OP = mybir.AluOpType

P = 128
EPS = 1e-5
DECAY = 16.0
CSCALE = 1.0


@dataclass
class Cfg:
    B: int = 4
    L: int = 2048
    E: int = 1024
    H: int = 16
    DH: int = 64
    F: int = 4096
    D: int = 6
    n_pair: int = 2  # cores per batch element

    @property
    def n_cores(self):
        return self.B * self.n_pair

    @property
    def R(self):
        return self.L // self.n_pair

    @property
    def LT(self):
        return self.R // P

    @property
    def MT(self):
        return self.L // P

    @property
    def ET(self):
        return self.E // P

    @property
    def FT(self):
        return self.F // P


FULL = Cfg()
TINY = Cfg(B=1, L=256, E=256, H=4, DH=64, F=512, D=2, n_pair=2)


def build_decoder(nc, cfg: Cfg, no_collective: bool = False):
    """Emit the per-core SPMD kernel. Returns nothing; declares DRAM I/O on nc."""
    c = cfg
    ISD = 1.0 / math.sqrt(c.DH)
    LT, MT, ET, FT = c.LT, c.MT, c.ET, c.FT
    L, R, E, F, H, D = c.L, c.R, c.E, c.F, c.H, c.D
    SC = min(512, L)          # matmul moving free dim for L-wide outputs
    NSC = L // SC
    SCq = min(512, R)
    NSCq = R // SCq
    SCCH = min(1024, L)       # scores psum chunk (<=2 banks)
    NCH = L // SCCH
    EH = min(512, E)          # sa e-half
    NEH = E // EH
    Lh = min(512, R)          # FFN l-half
    NLh = R // Lh
    FT2 = max(1, FT // 2)     # FFN2 f-half tiles
    NFh = FT // FT2
    BST = min(512, E)         # bn_stats chunk
    NST = E // BST
    HPT = P // c.DH           # heads per e'-tile (2)

    # ---- DRAM I/O ----
    xown_f32 = nc.dram_tensor("xown_f32", [P, LT, E], F32, kind="ExternalInput").ap()
    xown_bf = nc.dram_tensor("xown_bf", [R, E], BF16, kind="ExternalInput").ap()
    xfull_bf = nc.dram_tensor("xfull_bf", [L, E], BF16, kind="ExternalInput").ap()
    wqT_in = nc.dram_tensor("wqT", [D, P, ET, E], BF16, kind="ExternalInput").ap()
    wkT_in = nc.dram_tensor("wkT", [D, P, ET, E], BF16, kind="ExternalInput").ap()
    w1T_in = nc.dram_tensor("w1T", [D, P, ET, F], BF16, kind="ExternalInput").ap()
    w2T_in = nc.dram_tensor("w2T", [D, P, FT, E], BF16, kind="ExternalInput").ap()
    rel_in = nc.dram_tensor("relx", [LT, P, L], F32, kind="ExternalInput").ap()
    out_own = nc.dram_tensor("out_own", [P, LT, E], F32, kind="ExternalOutput").ap()

    groups = [[2 * b, 2 * b + 1] for b in range(c.B)] if c.n_pair == 2 else None

    from contextlib import ExitStack

    with tile.TileContext(nc) as tc, ExitStack() as ctx:
        singles = ctx.enter_context(tc.tile_pool(name="singles", bufs=1))
        dram = ctx.enter_context(tc.tile_pool(name="dram", bufs=1, space="DRAM"))
        ps_sc = ctx.enter_context(tc.tile_pool(name="ps_sc", bufs=2, space="PSUM"))
        ps_mm = ctx.enter_context(tc.tile_pool(name="ps_mm", bufs=4, space="PSUM"))
        epool = ctx.enter_context(tc.tile_pool(name="epool", bufs=3))
        accp = ctx.enter_context(tc.tile_pool(name="accp", bufs=1))
        wpool = ctx.enter_context(tc.tile_pool(name="wpool", bufs=2))
        hmtp = ctx.enter_context(tc.tile_pool(name="hmtp", bufs=2))
        h2p = ctx.enter_context(tc.tile_pool(name="h2p", bufs=3))
        smalls = ctx.enter_context(tc.tile_pool(name="smalls", bufs=2))

        # ---- persistent slabs (bf16 element = 2B) ----
        h_own = singles.tile([P, LT, E], F32, name="h_own")
        slabA = singles.tile([P, 2 * ET * E], BF16, name="slabA")
        #   views: wq/wk resident | zT_all | ff1T half
        wq_full = slabA[:, : ET * E].rearrange("p (a b) -> p a b", a=ET)
        wk_full = slabA[:, ET * E : 2 * ET * E].rearrange("p (a b) -> p a b", a=ET)
        zT_all = slabA[:, : MT * R].rearrange("p (a b) -> p a b", a=MT)
        ff1T = slabA[:, : FT * Lh].rearrange("p (a b) -> p a b", a=FT)
        slabB = singles.tile([P, ET * L], BF16, name="slabB")
        KT = slabB.rearrange("p (a b) -> p a b", a=ET)
        ffT_sb = slabB[:, : ET * R].rearrange("p (a b) -> p a b", a=ET)
        ff_row = slabB[:, ET * R : 2 * ET * R].rearrange("p (a b) -> p a b", a=LT)
        slabQ = singles.tile([P, ET * R], BF16, name="slabQ")
        QT = slabQ.rearrange("p (a b) -> p a b", a=ET)
        houtb = slabQ.rearrange("p (a b) -> p a b", a=LT)  # [P, LT, E]
        slabD = singles.tile([P, 2 * ET * SC], BF16, name="slabD")
        hT_chunk = [
            slabD[:, : ET * SC].rearrange("p (a b) -> p a b", a=ET),
            slabD[:, ET * SC : 2 * ET * SC].rearrange("p (a b) -> p a b", a=ET),
        ]
        h2T = slabD[:, : ET * R].rearrange("p (a b) -> p a b", a=ET)
        # slabC: 2 f32 [P, L] rel buffers (4L bf16 elems) + 1 bf16 [P, L] z buffer
        slabC = singles.tile([P, 5 * L], BF16, name="slabC")
        rb_f32 = [
            slabC[:, : 2 * L].bitcast(F32),
            slabC[:, 2 * L : 4 * L].bitcast(F32),
        ]  # two [P, L] f32 views (manual double buffer)
        z_bf = slabC[:, 4 * L : 5 * L]  # [P, L] bf16

        # persistent smalls
        recip2 = singles.tile([P, LT], F32, name="recip2")
        mv_all = singles.tile([P, LT, 2], F32, name="mv_all")
        rstd_all = singles.tile([P, LT], F32, name="rstd_all")
        rs2 = singles.tile([P, LT], F32, name="rs2")
        c_eps = singles.tile([P, 1], F32, name="c_eps")
        nc.vector.memset(c_eps, float(EPS))
        c_eps2 = singles.tile([P, 1], F32, name="c_eps2")
        nc.vector.memset(c_eps2, float(EPS * EPS))

        # DRAM exchange buffers
        hout_d = [
            dram.tile([R, E], BF16, name="hout0"),
            dram.tile([R, E], BF16, name="hout1"),
        ]
        hfull_d = [
            dram.tile([L, E], BF16, name="hfull0"),
            dram.tile([L, E], BF16, name="hfull1"),
        ]

        # init residual
        nc.sync.dma_start(out=h_own[:], in_=xown_f32[:])

        for d in range(D):
            hfull = xfull_bf if d == 0 else hfull_d[(d - 1) % 2]
            hown_prev = xown_bf if d == 0 else hout_d[(d - 1) % 2]

            # ---- load wq/wk resident ----
            for et in range(ET):
                nc.sync.dma_start(out=wq_full[:, et, :], in_=wqT_in[d, :, et, :])
                nc.sync.dma_start(out=wk_full[:, et, :], in_=wkT_in[d, :, et, :])

            # ---- K projection: KT[e', m] over full L ----
            for ci in range(NSC):
                hTc = hT_chunk[ci % 2]
                for et in range(ET):
                    nc.sync.dma_start_transpose(
                        hTc[:, et, :], hfull[ci * SC : (ci + 1) * SC, et * P : (et + 1) * P]
                    )
                for ept in range(ET):
                    ps = ps_mm.tile([P, 512], F32, tag="mm", name="ps_k")
                    for et in range(ET):
                        nc.tensor.matmul(
                            ps[:, :SC],
                            wk_full[:, et, ept * P : (ept + 1) * P],
                            hTc[:, et, :],
                            start=(et == 0),
                            stop=(et == ET - 1),
                        )
                    nc.vector.tensor_copy(
                        out=KT[:, ept, ci * SC : (ci + 1) * SC], in_=ps[:, :SC]
                    )

            # ---- Q projection: QT[e', l] own rows ----
            for ci in range(NSCq):
                hTc = hT_chunk[ci % 2]
                for et in range(ET):
                    nc.sync.dma_start_transpose(
                        hTc[:, et, :SCq],
                        hown_prev[ci * SCq : (ci + 1) * SCq, et * P : (et + 1) * P],
                    )
                for ept in range(ET):
                    ps = ps_mm.tile([P, 512], F32, tag="mm", name="ps_q")
                    for et in range(ET):
                        nc.tensor.matmul(
                            ps[:, :SCq],
                            wq_full[:, et, ept * P : (ept + 1) * P],
                            hTc[:, et, :SCq],
                            start=(et == 0),
                            stop=(et == ET - 1),
                        )
                    nc.vector.tensor_copy(
                        out=QT[:, ept, ci * SCq : (ci + 1) * SCq], in_=ps[:, :SCq]
                    )

            # ---- per l_tile scores/softmax in two groups; sa/res1/LN12 of a
            # group overlaps the next group's scores/softmax ----
            NG = 2 if LT >= 2 else 1
            GL = LT // NG
            for g in range(NG):
              for t in range(g * GL, (g + 1) * GL):
                rb = rb_f32[t % 2]
                nc.sync.dma_start(out=rb[:], in_=rel_in[t, :, :])
                rs_parts = smalls.tile([P, H, NCH], F32, tag="rsp", name="rs_parts")
                acc = accp.tile([P, L], BF16, tag="acc", name="acc")
                for h in range(H):
                    poff = c.DH * (h % HPT)
                    ept = h // HPT
                    qs = QT[poff : poff + c.DH, ept, t * P : (t + 1) * P]
                    e_pl = epool.tile([P, L], BF16, tag="epl", name="e_pl")
                    for ch in range(NCH):
                        pssc = ps_sc.tile([P, SCCH], F32, tag="sc", name="ps_sc")
                        for j in range(SCCH // SC):
                            m0 = ch * SCCH + j * SC
                            nc.tensor.matmul(
                                pssc[:, j * SC : (j + 1) * SC],
                                qs,
                                KT[poff : poff + c.DH, ept, m0 : m0 + SC],
                                start=True,
                                stop=True,
                            )
                        nc.scalar.activation(
                            out=e_pl[:, ch * SCCH : (ch + 1) * SCCH],
                            in_=pssc[:],
                            func=AF.Exp,
                            scale=ISD,
                            accum_out=rs_parts[:, h, ch : ch + 1],
                        )
                    # accumulate: acc += e_pl * (rsh/H); TS-mul (4x bf16) in
                    # place, then TT-add (2x bf16) -- cheaper than one 1x STT
                    rsh = smalls.tile([P, 1], F32, tag="rsh", name="rsh")
                    if NCH > 1:
                        nc.vector.tensor_reduce(
                            out=rsh, in_=rs_parts[:, h, :],
                            axis=mybir.AxisListType.X, op=OP.add,
                        )
                    else:
                        nc.vector.tensor_copy(out=rsh, in_=rs_parts[:, h, :])
                    nc.vector.reciprocal(out=rsh, in_=rsh)
                    nc.vector.tensor_scalar(
                        out=e_pl[:], in0=e_pl[:], scalar1=rsh,
                        scalar2=float(CSCALE / H), op0=OP.mult, op1=OP.mult,
                    )
                    if h == 0:
                        nc.vector.tensor_copy(out=acc[:], in_=e_pl[:])
                    else:
                        nc.vector.tensor_tensor(
                            out=acc[:], in0=e_pl[:], in1=acc[:], op=OP.add
                        )
                # stage 2: z = exp(acc + rel); s2 built in the f32 rel buffer
                nc.vector.tensor_tensor(
                    out=rb[:], in0=acc[:], in1=rb[:], op=OP.add
                )
                nc.scalar.activation(
                    out=z_bf[:], in_=rb[:], func=AF.Exp, scale=1.0,
                    accum_out=rs2[:, t : t + 1],
                )
                # transpose z into zT_all columns for this l_tile
                for mt in range(MT):
                    nc.sync.dma_start_transpose(
                        zT_all[:, mt, t * P : (t + 1) * P],
                        z_bf[:, mt * P : (mt + 1) * P],
                    )
              nc.vector.reciprocal(
                  out=recip2[:, g * GL : (g + 1) * GL],
                  in_=rs2[:, g * GL : (g + 1) * GL],
              )
              for eh in range(NEH):
                ps_sa = {}
                for t in range(g * GL, (g + 1) * GL):
                    ps_sa[t] = ps_mm.tile([P, 512], F32, tag="mm", name="ps_sa")[:, :EH]
                for mt in range(MT):
                    hmt = hmtp.tile([P, EH], BF16, tag="hmt", name="hmt")
                    nc.sync.dma_start(
                        out=hmt[:],
                        in_=hfull[mt * P : (mt + 1) * P, eh * EH : (eh + 1) * EH],
                    )
                    for t in range(g * GL, (g + 1) * GL):
                        nc.tensor.matmul(
                            ps_sa[t],
                            zT_all[:, mt, t * P : (t + 1) * P],
                            hmt[:],
                            start=(mt == 0),
                            stop=(mt == MT - 1),
                        )
                for t in range(g * GL, (g + 1) * GL):
                    # res1 = h + sa*recip2  (in place on h_own)
                    nc.vector.scalar_tensor_tensor(
                        out=h_own[:, t, eh * EH : (eh + 1) * EH],
                        in0=ps_sa[t],
                        scalar=recip2[:, t : t + 1],
                        in1=h_own[:, t, eh * EH : (eh + 1) * EH],
                        op0=OP.mult,
                        op1=OP.add,
                    )

              # LN1+LN2 fused: h2 = (res1 - m) / sqrt(v*(1+eps) + eps^2)
              for t in range(g * GL, (g + 1) * GL):
                  stats = smalls.tile([P, NST, 6], F32, tag="st", name="stats")
                  for i in range(NST):
                      nc.vector.bn_stats(
                          out=stats[:, i, :], in_=h_own[:, t, i * BST : (i + 1) * BST]
                      )
                  mv = mv_all[:, t, :]
                  nc.vector.bn_aggr(out=mv, in_=stats[:])
                  sq = rstd_all[:, t : t + 1]
                  nc.scalar.activation(
                      out=sq, in_=mv_all[:, t, 1:2], func=AF.Sqrt,
                      bias=c_eps2, scale=float(1.0 + EPS),
                  )
                  nc.vector.reciprocal(out=sq, in_=sq)
                  h2st = h2p.tile([P, E], BF16, tag="h2st", name="h2st")
                  nc.vector.tensor_scalar(
                      out=h2st[:], in0=h_own[:, t, :],
                      scalar1=mv_all[:, t, 0:1], scalar2=sq,
                      op0=OP.subtract, op1=OP.mult,
                  )
                  for et in range(ET):
                      nc.sync.dma_start_transpose(
                          h2T[:, et, t * P : (t + 1) * P],
                          h2st[:, et * P : (et + 1) * P],
                      )

            # ---- FFN ----
            for lh in range(NLh):
                for ft in range(FT):
                    w1b = wpool.tile([P, ET, P], BF16, tag="w1", name="w1b")
                    nc.sync.dma_start(out=w1b[:], in_=w1T_in[d, :, :, ft * P : (ft + 1) * P])
                    ps = ps_mm.tile([P, 512], F32, tag="mm", name="ps_f1")
                    for et in range(ET):
                        nc.tensor.matmul(
                            ps[:, :Lh],
                            w1b[:, et, :],
                            h2T[:, et, lh * Lh : (lh + 1) * Lh],
                            start=(et == 0),
                            stop=(et == ET - 1),
                        )
                    nc.vector.tensor_scalar(
                        out=ff1T[:, ft, :Lh], in0=ps[:, :Lh], scalar1=0.0, scalar2=None,
                        op0=OP.max,
                    )
                for ept in range(ET):
                    ps2 = ps_mm.tile([P, 512], F32, tag="mm", name="ps_f2")
                    for fh in range(NFh):
                        w2b = wpool.tile([P, FT2, P], BF16, tag="w2", name="w2b")
                        nc.sync.dma_start(
                            out=w2b[:],
                            in_=w2T_in[d, :, fh * FT2 : (fh + 1) * FT2, ept * P : (ept + 1) * P],
                        )
                        for f2 in range(FT2):
                            nc.tensor.matmul(
                                ps2[:, :Lh],
                                w2b[:, f2, :],
                                ff1T[:, fh * FT2 + f2, :Lh],
                                start=(fh == 0 and f2 == 0),
                                stop=(fh == NFh - 1 and f2 == FT2 - 1),
                            )
                    nc.vector.tensor_copy(
                        out=ffT_sb[:, ept, lh * Lh : (lh + 1) * Lh], in_=ps2[:, :Lh]
                    )
            # ff transposes -> ff_row
            for t in range(LT):
                for et in range(ET):
                    nc.sync.dma_start_transpose(
                        ff_row[:, t, et * P : (et + 1) * P],
                        ffT_sb[:, et, t * P : (t + 1) * P],
                    )

            # ---- res2 + LN3 ----
            last = d == D - 1
            for t in range(LT):
                h2mt = h2p.tile([P, E], BF16, tag="h2mt", name="h2mt")
                nc.vector.tensor_scalar(
                    out=h2mt[:], in0=h_own[:, t, :],
                    scalar1=mv_all[:, t, 0:1], scalar2=rstd_all[:, t : t + 1],
                    op0=OP.subtract, op1=OP.mult,
                )
                nc.vector.scalar_tensor_tensor(
                    out=h_own[:, t, :], in0=ff_row[:, t, :], scalar=1.0,
                    in1=h2mt[:], op0=OP.mult, op1=OP.add,
                )
                stats = smalls.tile([P, NST, 6], F32, tag="st", name="stats3")
                for i in range(NST):
                    nc.vector.bn_stats(
                        out=stats[:, i, :], in_=h_own[:, t, i * BST : (i + 1) * BST]
                    )
                mv = smalls.tile([P, 2], F32, tag="mv", name="mv3")
                nc.vector.bn_aggr(out=mv[:], in_=stats[:])
                sq = smalls.tile([P, 1], F32, tag="sq", name="sq3")
                nc.scalar.activation(
                    out=sq, in_=mv[:, 1:2], func=AF.Sqrt, bias=c_eps, scale=1.0
                )
                nc.vector.reciprocal(out=sq, in_=sq)
                nc.vector.tensor_scalar(
                    out=h_own[:, t, :], in0=h_own[:, t, :],
                    scalar1=mv[:, 0:1], scalar2=sq, op0=OP.subtract, op1=OP.mult,
                )
                if not last:
                    nc.vector.tensor_copy(out=houtb[:, t, :], in_=h_own[:, t, :])
                    nc.sync.dma_start(
                        out=hout_d[d % 2][t * P : (t + 1) * P, :],
                        in_=houtb[:, t, :],
                    )
            if not last and no_collective:
                # timing-only single-core stand-in for the AllGather
                nc.sync.dma_start(out=hfull_d[d % 2][:R], in_=hout_d[d % 2][:])
                nc.sync.dma_start(out=hfull_d[d % 2][R:], in_=hout_d[d % 2][:])
            elif not last and groups is not None:
                nc.gpsimd.collective_compute(
                    "AllGather",
                    OP.bypass,
                    replica_groups=groups,
                    ins=[hout_d[d % 2].opt()],
                    outs=[hfull_d[d % 2].opt()],
                )
            elif not last:
                # single-pair-less config (n_pair==1): copy own -> full
                nc.sync.dma_start(out=hfull_d[d % 2][:], in_=hout_d[d % 2][:])

        nc.sync.dma_start(out=out_own[:], in_=h_own[:])


# ---------------- host-side helpers ----------------

def make_rel(L):
    pos = np.arange(L)
    return np.exp(-np.abs(pos[:, None] - pos[None, :]).astype(np.float32) / DECAY)


def prep_inputs(cfg: Cfg, inputs):
    """inputs: dict of full numpy arrays as in reference.setup_inputs().
    Returns list of per-core in_maps."""
    c = cfg
    x = np.asarray(inputs["x"], np.float32)
    Wq = np.asarray(inputs["Wq"], np.float32)
    Wk = np.asarray(inputs["Wk"], np.float32)
    W1 = np.asarray(inputs["W1"], np.float32)
    W2 = np.asarray(inputs["W2"], np.float32)
    rel = make_rel(c.L)

    def to_lhsT(w):  # [D, out, in] -> [D, P, in_tiles, out] (w.T tiled on partitions)
        D_, O_, I_ = w.shape
        wT = np.ascontiguousarray(np.transpose(w, (0, 2, 1)))  # [D, in, out]
        return wT.reshape(D_, I_ // P, P, O_).transpose(0, 2, 1, 3).astype(ml_bf16())

    wqT = to_lhsT(Wq)
    wkT = to_lhsT(Wk)
    w1T = to_lhsT(W1)
    w2T = to_lhsT(W2)

    in_maps = []
    for core in range(c.n_cores):
        b = core // c.n_pair
        s = core % c.n_pair
        R0 = s * c.R
        xrows = x[b, R0 : R0 + c.R]  # [R, E]
        xown_f32 = np.ascontiguousarray(
            xrows.reshape(c.LT, P, c.E).transpose(1, 0, 2)
        )
        relx = np.ascontiguousarray(
            rel[R0 : R0 + c.R].reshape(c.LT, P, c.L)
        )
        in_maps.append(
            {
                "xown_f32": xown_f32,
                "xown_bf": xrows.astype(ml_bf16()),
                "xfull_bf": x[b].astype(ml_bf16()),
                "wqT": wqT,
                "wkT": wkT,
                "w1T": w1T,
                "w2T": w2T,
                "relx": relx,
            }
        )
    return in_maps


def assemble(cfg: Cfg, results):
    """results: list of per-core {'out_own': [P, LT, E]} -> full [B, L, E] f32."""
    c = cfg
    out = np.zeros((c.B, c.L, c.E), np.float32)
    for core in range(c.n_cores):
        b = core // c.n_pair
        s = core % c.n_pair
        R0 = s * c.R
        oo = results[core]["out_own"]  # [P, LT, E]
        out[b, R0 : R0 + c.R] = oo.transpose(1, 0, 2).reshape(c.R, c.E)
    return out


def ml_bf16():
    import ml_dtypes

    return ml_dtypes.bfloat16


# ---------------- public entry ----------------

_CACHE = {}


def _get_nc(cfg: Cfg):
    key = ("nc", cfg.L, cfg.D)
    if key not in _CACHE:
        import concourse.bacc as bacc

        nc = bacc.Bacc(
            "TRN2", target_bir_lowering=False, debug=False, num_devices=cfg.n_cores
        )
        build_decoder(nc, cfg)
        nc.compile()
        _CACHE[key] = nc
    return _CACHE[key]


def run(inputs, cfg: Cfg = FULL, trace: bool = False, **spmd_kwargs):
    from concourse.bass_utils import run_bass_kernel_spmd

    nc = _get_nc(cfg)
    in_maps = prep_inputs(cfg, inputs)
    res = run_bass_kernel_spmd(
        nc, in_maps, core_ids=list(range(cfg.n_cores)), trace=trace, **spmd_kwargs
    )
    out = assemble(cfg, res.results)
    return out, res


def kernel(**inputs):
    out, _ = run(inputs)
    return out.astype(np.float32)



# revision 7
# speedup vs baseline: 1.0642x; 1.0642x over previous
"""Trainium2 Bass kernel for nn_Decoder_11278584119887 (self-contained).

6-layer dense transformer decoder with head-averaged attention weights.
Sharding: 8 NeuronCores = 4 batch elements x 2 sequence halves; per-layer
bf16 AllGather (pairs), split into two halves so each half can overlap
compute.

Layout note: the m (key/value position) axis is kept in CORE-LOCAL order
[gatherA | gatherB] where gatherA = [core0 rows 0:RH | core1 rows 0:RH]
and RH = R/2.  This makes every DRAM source core-independent (SPMD safe).
rel columns and the layer-0 full input are host-permuted to match; the
softmax/sa math is invariant to a column permutation.

v2 vs baseline:
 - bulk DMAs moved to the Pool (SWDGE) queue + batched; transposes batched
   (one per chunk / l_tile); weights repacked host-side for contiguous
   reads.  (TimelineSim showed the SP dispatch queue saturated.)
 - Wq/Wk projections in fp8 e4m3 with DoubleRow (safe: w_avg only perturbs
   softmax(rel) weights by ~1e-3 multiplicatively).
 - scores head-pair matmuls interleaved for 64-row PE-tile concurrency.
 - rel bf16, preloaded into the softmax accumulator (saves the stage-2 add).
 - FFN emitted per sequence-half inside the scores sweep; LN3 + AllGather
   per half issued as soon as the half's rows are done.
"""

import sys as _sys

for _p in ("/opt/trn_rl_repo",):
    if _p not in _sys.path:
        _sys.path.insert(0, _p)

import math
from contextlib import ExitStack
from dataclasses import dataclass

import numpy as np

import concourse.bass as bass
import concourse.mybir as mybir
import concourse.tile as tile

F32 = mybir.dt.float32
BF16 = mybir.dt.bfloat16
FP8 = mybir.dt.float8e4
AF = mybir.ActivationFunctionType
OP = mybir.AluOpType
DR = mybir.MatmulPerfMode.DoubleRow

P = 128
EPS = 1e-5
DECAY = 16.0
CSCALE = 1.0


@dataclass
class Cfg:
    B: int = 4
    L: int = 2048
    E: int = 1024
    H: int = 16
    DH: int = 64
    F: int = 4096
    D: int = 6
    n_pair: int = 2

    @property
    def n_cores(self):
        return self.B * self.n_pair

    @property
    def R(self):
        return self.L // self.n_pair

    @property
    def RH(self):
        return self.R // 2

    @property
    def LT(self):
        return self.R // P

    @property
    def MT(self):
        return self.L // P

    @property
    def ET(self):
        return self.E // P

    @property
    def FT(self):
        return self.F // P


FULL = Cfg()
TINY = Cfg(B=1, L=512, E=512, H=8, DH=64, F=1024, D=2, n_pair=2)


def build_decoder(nc, cfg: Cfg, no_collective: bool = False, use_dr: bool = True):
    c = cfg
    ISD = 1.0 / math.sqrt(c.DH)
    LT, MT, ET, FT = c.LT, c.MT, c.ET, c.FT
    L, R, E, F, H, D, RH = c.L, c.R, c.E, c.F, c.H, c.D, c.RH
    SC = RH                      # hT chunk rows; must divide gather halves
    assert SC <= 512
    NSC = L // SC
    NOWN = R // SC               # own chunks (for Q)
    SCCH = min(1024, L)          # scores psum chunk (<=2 banks)
    NCH = L // SCCH
    SCJ = min(512, SCCH)
    NJ = SCCH // SCJ
    EH = min(512, E)
    NEH = E // EH
    GL = min(2, LT)              # l_tiles per sa group
    Lh = min(512, R)             # FFN row half
    NLh = R // Lh
    LhT = Lh // P
    FT2 = min(8, FT)             # W2 f-tiles per streamed chunk
    NFh = FT // FT2
    BST = min(512, E)
    NST = E // BST
    NPAIR = H // 2
    NB = 2 if 2 * P <= 2 * RH else 1   # m-tiles per sa hmt load
    DHH = c.DH

    # ---- DRAM I/O ----
    xown_f32 = nc.dram_tensor("xown_f32", [P, LT, E], F32, kind="ExternalInput").ap()
    xown_bf = nc.dram_tensor("xown_bf", [R, E], BF16, kind="ExternalInput").ap()
    xfull_bf = nc.dram_tensor("xfull_bf", [L, E], BF16, kind="ExternalInput").ap()
    wq8_in = nc.dram_tensor("wq8", [D, P, ET, E], FP8, kind="ExternalInput").ap()
    wk8_in = nc.dram_tensor("wk8", [D, P, ET, E], FP8, kind="ExternalInput").ap()
    w1r_in = nc.dram_tensor("w1r", [D, P, FT, ET * P], BF16, kind="ExternalInput").ap()
    w2r_in = nc.dram_tensor("w2r", [D, P, ET, FT * P], BF16, kind="ExternalInput").ap()
    rel_in = nc.dram_tensor("relx", [LT, P, L], BF16, kind="ExternalInput").ap()
    out_own = nc.dram_tensor("out_own", [P, LT, E], F32, kind="ExternalOutput").ap()

    groups = [[2 * b, 2 * b + 1] for b in range(c.B)] if c.n_pair == 2 else None

    with tile.TileContext(nc) as tc, ExitStack() as ctx:
        singles = ctx.enter_context(tc.tile_pool(name="singles", bufs=1))
        dram = ctx.enter_context(tc.tile_pool(name="dram", bufs=1, space="DRAM"))
        ps_sc = ctx.enter_context(tc.tile_pool(name="ps_sc", bufs=2, space="PSUM"))
        ps_mm = ctx.enter_context(tc.tile_pool(name="ps_mm", bufs=2, space="PSUM"))
        ps_sa = ctx.enter_context(tc.tile_pool(name="ps_sa", bufs=2, space="PSUM"))
        smalls = ctx.enter_context(tc.tile_pool(name="smalls", bufs=2))

        # ---- persistent SBUF slabs ----
        h_own = singles.tile([P, LT, E], F32, name="h_own")
        KT = singles.tile([P, ET, L], BF16, name="KT")
        slabQ = singles.tile([P, ET * R], BF16, name="slabQ")
        QT = slabQ.rearrange("p (a b) -> p a b", a=ET)
        houtb = slabQ.rearrange("p (a b) -> p a b", a=LT)
        zTg = singles.tile([P, GL, MT, P], BF16, name="zTg")
        # sweep buffers; wq8/wk8 alias them (weights dead before the sweep)
        swslab = singles.tile([P, 4 * L], BF16, name="swslab")
        racc = [swslab[:, i * L : (i + 1) * L] for i in range(2)]
        e_pl = [swslab[:, (2 + i) * L : (3 + i) * L] for i in range(2)]
        wq8 = swslab[:, : ET * E // 2].bitcast(FP8).rearrange(
            "p (a b) -> p a b", a=ET
        )
        wk8 = swslab[:, ET * E // 2 : ET * E].bitcast(FP8).rearrange(
            "p (a b) -> p a b", a=ET
        )
        z_bf = singles.tile([P, L], BF16, name="z_bf")
        # hT chunks (bf16 + fp8); h2T overlays the bf16 chunk region
        slabD = singles.tile([P, 3 * ET * SC], BF16, name="slabD")
        hTb = [
            slabD[:, : ET * SC].rearrange("p (a b) -> p a b", a=ET),
            slabD[:, ET * SC : 2 * ET * SC].rearrange("p (a b) -> p a b", a=ET),
        ]
        h2T = slabD[:, : LT * ET * P].rearrange("p (t a b) -> p t a b", t=LT, a=ET)
        hT8 = [
            slabD[:, 2 * ET * SC : 2 * ET * SC + ET * SC // 2]
            .bitcast(FP8)
            .rearrange("p (a b) -> p a b", a=ET),
            slabD[:, 2 * ET * SC + ET * SC // 2 : 3 * ET * SC]
            .bitcast(FP8)
            .rearrange("p (a b) -> p a b", a=ET),
        ]
        # FFN slabs
        ff1T = singles.tile([P, FT, Lh], BF16, name="ff1T")
        ffT_sb = singles.tile([P, ET, Lh], BF16, name="ffT_sb")
        ff_rowT = singles.tile([P, ET * LhT, P], BF16, name="ff_rowT")
        ffv = ff_rowT.rearrange("p (a t) c -> p a t c", a=ET)
        # weight streams + hmt + h2st: manual double buffers in one slab
        wslab = singles.tile([P, 2 * ET * P + 2 * FT2 * P], BF16, name="wslab")
        w1b = [
            wslab[:, i * ET * P : (i + 1) * ET * P].rearrange(
                "p (a b) -> p a b", a=ET
            )
            for i in range(2)
        ]
        w2b = [
            wslab[
                :, 2 * ET * P + i * FT2 * P : 2 * ET * P + (i + 1) * FT2 * P
            ].rearrange("p (a b) -> p a b", a=FT2)
            for i in range(2)
        ]
        hmslab = singles.tile([P, 2 * NB * EH], BF16, name="hmslab")
        hmt = [
            hmslab[:, i * NB * EH : (i + 1) * NB * EH].rearrange(
                "p (a b) -> p a b", a=NB
            )
            for i in range(2)
        ]
        h2slab = singles.tile([P, 2 * E], BF16, name="h2slab")
        h2st = [h2slab[:, i * E : (i + 1) * E] for i in range(2)]
        # smalls packed into one f32 slab
        sm = singles.tile([P, 256], F32, name="sm")
        recip2 = sm[:, 0:LT]
        rs2 = sm[:, 16 : 16 + LT]
        c_eps = sm[:, 32:33]
        c_eps2 = sm[:, 33:34]
        rsh_s = [sm[:, 34 + i : 35 + i] for i in range(4)]
        mvt = sm[:, 40:42]
        sqt = sm[:, 42:43]
        st12 = sm[:, 48 : 48 + NST * 6].rearrange("p (a b) -> p a b", a=NST)
        mv3 = sm[:, 84:86]
        sq3 = sm[:, 86:87]
        st3 = sm[:, 88 : 88 + NST * 6].rearrange("p (a b) -> p a b", a=NST)
        rsp_b = [
            sm[:, 128 + i * H * NCH : 128 + (i + 1) * H * NCH].rearrange(
                "p (a b) -> p a b", a=H
            )
            for i in range(2)
        ]
        nc.vector.memset(c_eps, float(EPS))
        nc.vector.memset(c_eps2, float(EPS * EPS))

        # ---- DRAM exchange buffers (per parity, per half) ----
        hout_d = [
            [dram.tile([RH, E], BF16, name=f"hout{i}{j}") for j in range(2)]
            for i in range(2)
        ]
        hhalf_d = [
            [
                dram.tile([c.n_pair * RH, E], BF16, name=f"hh{i}{j}")
                for j in range(2)
            ]
            for i in range(2)
        ]

        def gcols(d, r0, nrows):
            """DRAM rows for local m-columns [r0, r0+nrows) of layer d input."""
            if d == 0:
                return xfull_bf[r0 : r0 + nrows]
            half = r0 // (c.n_pair * RH)
            off = r0 - half * c.n_pair * RH
            return hhalf_d[(d - 1) % 2][half][off : off + nrows]

        def own_rows(d, r0, nrows):
            """Own rows (local l index) of layer d input."""
            if d == 0:
                return xown_bf[r0 : r0 + nrows]
            half = r0 // RH
            off = r0 - half * RH
            return hout_d[(d - 1) % 2][half][off : off + nrows]

        nc.gpsimd.dma_start(out=h_own[:], in_=xown_f32[:])

        def proj(idx, buf, src, wt, dst):
            """One hT chunk: transpose src rows -> fp8 -> project with wt
            into dst columns [idx*SC, (idx+1)*SC)."""
            nc.sync.dma_start_transpose(hTb[buf][:], src)
            nc.scalar.activation(
                out=hT8[buf][:], in_=hTb[buf][:], func=AF.Copy, scale=1.0
            )
            for ept in range(ET):
                ps = ps_mm.tile([P, 512], F32, tag="mm", name="ps_p")
                if use_dr and ET % 2 == 0:
                    for jp in range(ET // 2):
                        nc.tensor.matmul(
                            ps[:, :SC],
                            wt[:, 2 * jp : 2 * jp + 2, ept * P : (ept + 1) * P],
                            hT8[buf][:, 2 * jp : 2 * jp + 2, :],
                            start=(jp == 0),
                            stop=(jp == ET // 2 - 1),
                            perf_mode=DR,
                        )
                else:
                    for et in range(ET):
                        nc.tensor.matmul(
                            ps[:, :SC],
                            wt[:, et, ept * P : (ept + 1) * P],
                            hT8[buf][:, et, :],
                            start=(et == 0),
                            stop=(et == ET - 1),
                        )
                nc.vector.tensor_copy(
                    out=dst[:, ept, idx * SC : (idx + 1) * SC], in_=ps[:, :SC]
                )

        def ffn_half(d, lh):
            """FFN + LN3 for l_tiles [lh*LhT, (lh+1)*LhT); store + collective
            for any fully-finished row half."""
            last = d == D - 1
            with nc.named_scope(f"d{d}ffn{lh}"):
                for ft in range(FT):
                    wb = w1b[ft % 2]
                    nc.gpsimd.dma_start(out=wb[:], in_=w1r_in[d, :, ft])
                    ps = ps_mm.tile([P, 512], F32, tag="mm", name="ps_f1")
                    for et in range(ET):
                        nc.tensor.matmul(
                            ps[:, :Lh],
                            wb[:, et, :],
                            h2T[:, lh * LhT : (lh + 1) * LhT, et, :],
                            start=(et == 0),
                            stop=(et == ET - 1),
                        )
                    nc.vector.tensor_scalar(
                        out=ff1T[:, ft, :], in0=ps[:, :Lh], scalar1=0.0,
                        scalar2=None, op0=OP.max,
                    )
                for ept in range(ET):
                    ps2 = ps_mm.tile([P, 512], F32, tag="mm", name="ps_f2")
                    for fh in range(NFh):
                        wb2 = w2b[fh % 2]
                        nc.gpsimd.dma_start(
                            out=wb2[:],
                            in_=w2r_in[d, :, ept, fh * FT2 * P : (fh + 1) * FT2 * P],
                        )
                        for f2 in range(FT2):
                            nc.tensor.matmul(
                                ps2[:, :Lh],
                                wb2[:, f2, :],
                                ff1T[:, fh * FT2 + f2, :],
                                start=(fh == 0 and f2 == 0),
                                stop=(fh == NFh - 1 and f2 == FT2 - 1),
                            )
                    nc.vector.tensor_copy(out=ffT_sb[:, ept, :], in_=ps2[:, :Lh])
                nc.sync.dma_start_transpose(ff_rowT[:], ffT_sb[:])
                # res2 + LN3 per tile of this half
                for tl in range(LhT):
                    tt = lh * LhT + tl
                    nc.vector.scalar_tensor_tensor(
                        out=h_own[:, tt, :],
                        in0=ffv[:, :, tl, :],
                        scalar=1.0,
                        in1=h_own[:, tt, :],
                        op0=OP.mult,
                        op1=OP.add,
                    )
                    for i in range(NST):
                        nc.vector.bn_stats(
                            out=st3[:, i, :],
                            in_=h_own[:, tt, i * BST : (i + 1) * BST],
                        )
                    nc.vector.bn_aggr(out=mv3[:], in_=st3[:])
                    nc.scalar.activation(
                        out=sq3, in_=mv3[:, 1:2], func=AF.Sqrt, bias=c_eps, scale=1.0
                    )
                    nc.vector.reciprocal(out=sq3, in_=sq3)
                    nc.vector.tensor_scalar(
                        out=h_own[:, tt, :], in0=h_own[:, tt, :],
                        scalar1=mv3[:, 0:1], scalar2=sq3,
                        op0=OP.subtract, op1=OP.mult,
                    )
                    if not last:
                        nc.vector.tensor_copy(
                            out=houtb[:, tt, :], in_=h_own[:, tt, :]
                        )
                if last:
                    return
                # store + collective for finished halves
                assert RH >= P
                done_rows = (lh + 1) * LhT * P
                for hf in range(2):
                    lo, hi = hf * RH, (hf + 1) * RH
                    if done_rows < hi or lh * LhT * P >= hi:
                        continue
                    dst = hout_d[d % 2][hf]
                    tpr = RH // P
                    nc.gpsimd.dma_start(
                        out=dst.rearrange("(t p) e -> p t e", p=P),
                        in_=houtb[:, hf * tpr : (hf + 1) * tpr, :],
                    )
                    if no_collective:
                        hh = hhalf_d[d % 2][hf]
                        nc.sync.dma_start(out=hh[:RH], in_=dst[:])
                        nc.sync.dma_start(out=hh[RH:], in_=dst[:])
                    else:
                        nc.gpsimd.collective_compute(
                            "AllGather",
                            OP.bypass,
                            replica_groups=groups,
                            ins=[dst.opt()],
                            outs=[hhalf_d[d % 2][hf].opt()],
                        )

        for d in range(D):
            last = d == D - 1
            with nc.named_scope(f"d{d}proj"):
                nc.gpsimd.dma_start(out=wq8[:], in_=wq8_in[d])
                nc.gpsimd.dma_start(out=wk8[:], in_=wk8_in[d])
                # Q chunks first (own rows, no collective dependency)
                for qi in range(NOWN):
                    proj(qi, qi % 2, own_rows(d, qi * SC, SC), wq8, QT)
                for ci in range(NSC):
                    proj(ci, ci % 2, gcols(d, ci * SC, SC), wk8, KT)

            # ---- scores sweep ----
            for t in range(LT):
                with nc.named_scope(f"d{d}sw{t}"):
                    rb = racc[t % 2]
                    nc.gpsimd.dma_start(out=rb[:], in_=rel_in[t])
                    rsp = rsp_b[t % 2]
                    for q in range(NPAIR):
                        h0, h1 = 2 * q, 2 * q + 1
                        for ch in range(NCH):
                            pA = ps_sc.tile([P, SCCH], F32, tag="sc", name="psA")
                            pB = ps_sc.tile([P, SCCH], F32, tag="sc", name="psB")
                            for j in range(NJ):
                                m0 = ch * SCCH + j * SCJ
                                for hh, pp in ((h0, pA), (h1, pB)):
                                    pof = DHH * (hh % 2)
                                    nc.tensor.matmul(
                                        pp[:, j * SCJ : (j + 1) * SCJ],
                                        QT[pof : pof + DHH, hh // 2,
                                           t * P : (t + 1) * P],
                                        KT[pof : pof + DHH, hh // 2,
                                           m0 : m0 + SCJ],
                                        start=True,
                                        stop=True,
                                    )
                            for hh, pp in ((h0, pA), (h1, pB)):
                                nc.scalar.activation(
                                    out=e_pl[hh % 2][:, ch * SCCH : (ch + 1) * SCCH],
                                    in_=pp[:],
                                    func=AF.Exp,
                                    scale=ISD,
                                    accum_out=rsp[:, hh, ch : ch + 1],
                                )
                        for hh in (h0, h1):
                            eb = e_pl[hh % 2]
                            rsh = rsh_s[hh % 4]
                            if NCH > 1:
                                nc.vector.tensor_reduce(
                                    out=rsh, in_=rsp[:, hh, :],
                                    axis=mybir.AxisListType.X, op=OP.add,
                                )
                                nc.vector.reciprocal(out=rsh, in_=rsh)
                            else:
                                nc.vector.reciprocal(out=rsh, in_=rsp[:, hh, :])
                            nc.vector.tensor_scalar(
                                out=eb[:], in0=eb[:], scalar1=rsh,
                                scalar2=float(CSCALE / H),
                                op0=OP.mult, op1=OP.mult,
                            )
                            nc.vector.tensor_tensor(
                                out=rb[:], in0=eb[:], in1=rb[:], op=OP.add
                            )
                    nc.scalar.activation(
                        out=z_bf[:], in_=rb[:], func=AF.Exp, scale=1.0,
                        accum_out=rs2[:, t : t + 1],
                    )
                    nc.sync.dma_start_transpose(zTg[:, t % GL], z_bf[:])

                if t % GL == GL - 1:
                    g = t // GL
                    t0 = g * GL
                    with nc.named_scope(f"d{d}sa{g}"):
                        nc.vector.reciprocal(
                            out=recip2[:, t0 : t0 + GL], in_=rs2[:, t0 : t0 + GL]
                        )
                        for eh in range(NEH):
                            pss = [
                                ps_sa.tile([P, EH], F32, tag="sa", name="ps_sa")
                                for _ in range(GL)
                            ]
                            for mi, ms in enumerate(range(0, MT, NB)):
                                hb = hmt[mi % 2]
                                nc.gpsimd.dma_start(
                                    out=hb[:],
                                    in_=gcols(d, ms * P, NB * P).rearrange(
                                        "(a p) e -> p a e", p=P
                                    )[:, :, eh * EH : (eh + 1) * EH],
                                )
                                for tg in range(GL):
                                    for k in range(NB):
                                        nc.tensor.matmul(
                                            pss[tg],
                                            zTg[:, tg, ms + k, :],
                                            hb[:, k, :],
                                            start=(mi == 0 and k == 0),
                                            stop=(ms + k == MT - 1),
                                        )
                            for tg in range(GL):
                                nc.vector.scalar_tensor_tensor(
                                    out=h_own[:, t0 + tg, eh * EH : (eh + 1) * EH],
                                    in0=pss[tg],
                                    scalar=recip2[:, t0 + tg : t0 + tg + 1],
                                    in1=h_own[:, t0 + tg, eh * EH : (eh + 1) * EH],
                                    op0=OP.mult,
                                    op1=OP.add,
                                )
                        # LN1+LN2 fused, h2 in place (f32) + bf16 copy + hT
                        for tg in range(GL):
                            tt = t0 + tg
                            for i in range(NST):
                                nc.vector.bn_stats(
                                    out=st12[:, i, :],
                                    in_=h_own[:, tt, i * BST : (i + 1) * BST],
                                )
                            nc.vector.bn_aggr(out=mvt[:], in_=st12[:])
                            nc.scalar.activation(
                                out=sqt, in_=mvt[:, 1:2], func=AF.Sqrt,
                                bias=c_eps2, scale=float(1.0 + EPS),
                            )
                            nc.vector.reciprocal(out=sqt, in_=sqt)
                            nc.vector.tensor_scalar(
                                out=h_own[:, tt, :], in0=h_own[:, tt, :],
                                scalar1=mvt[:, 0:1], scalar2=sqt,
                                op0=OP.subtract, op1=OP.mult,
                            )
                            hs = h2st[tg % 2]
                            nc.vector.tensor_copy(out=hs[:], in_=h_own[:, tt, :])
                            nc.sync.dma_start_transpose(h2T[:, tt], hs[:])
                    # FFN for any completed row half
                    for lh in range(NLh):
                        if (lh + 1) * Lh == (t + 1) * P:
                            ffn_half(d, lh)

        nc.gpsimd.dma_start(out=out_own[:], in_=h_own[:])


# ---------------- host-side helpers ----------------


def make_rel(L):
    pos = np.arange(L)
    return np.exp(-np.abs(pos[:, None] - pos[None, :]).astype(np.float32) / DECAY)


def perm_cols(c: Cfg):
    """Core-local m order: [gatherA | gatherB] halves."""
    L, R, RH = c.L, c.R, c.RH
    return np.concatenate(
        [
            np.arange(0, RH),
            np.arange(R, R + RH),
            np.arange(RH, R),
            np.arange(R + RH, L),
        ]
    )


def ml_bf16():
    import ml_dtypes

    return ml_dtypes.bfloat16


def ml_fp8():
    import ml_dtypes

    return ml_dtypes.float8_e4m3


def prep_inputs(cfg: Cfg, inputs):
    c = cfg
    x = np.asarray(inputs["x"], np.float32)
    Wq = np.asarray(inputs["Wq"], np.float32)
    Wk = np.asarray(inputs["Wk"], np.float32)
    W1 = np.asarray(inputs["W1"], np.float32)
    W2 = np.asarray(inputs["W2"], np.float32)
    rel = make_rel(c.L)
    pc = perm_cols(c)

    def to_lhsT(w):  # [D, out, in] -> [D, P, in_tiles, out]
        D_, O_, I_ = w.shape
        wT = np.ascontiguousarray(np.transpose(w, (0, 2, 1)))
        return np.ascontiguousarray(
            wT.reshape(D_, I_ // P, P, O_).transpose(0, 2, 1, 3)
        )

    wq8 = to_lhsT(Wq).astype(ml_fp8())
    wk8 = to_lhsT(Wk).astype(ml_fp8())
    w1T = to_lhsT(W1)  # [D, P, ET, F]
    D_, _, ET_, F_ = w1T.shape
    FT_ = F_ // P
    w1r = np.ascontiguousarray(
        w1T.reshape(D_, P, ET_, FT_, P).transpose(0, 1, 3, 2, 4)
    ).reshape(D_, P, FT_, ET_ * P).astype(ml_bf16())
    w2T = to_lhsT(W2)  # [D, P, FT, E]
    _, _, FT2_, E_ = w2T.shape
    ET2_ = E_ // P
    w2r = np.ascontiguousarray(
        w2T.reshape(D_, P, FT2_, ET2_, P).transpose(0, 1, 3, 2, 4)
    ).reshape(D_, P, ET2_, FT2_ * P).astype(ml_bf16())

    in_maps = []
    for core in range(c.n_cores):
        b = core // c.n_pair
        s = core % c.n_pair
        R0 = s * c.R
        xrows = x[b, R0 : R0 + c.R]
        xown_f32 = np.ascontiguousarray(xrows.reshape(c.LT, P, c.E).transpose(1, 0, 2))
        relx = np.ascontiguousarray(
            rel[R0 : R0 + c.R][:, pc].reshape(c.LT, P, c.L)
        ).astype(ml_bf16())
        in_maps.append(
            {
                "xown_f32": xown_f32,
                "xown_bf": xrows.astype(ml_bf16()),
                "xfull_bf": np.ascontiguousarray(x[b][pc]).astype(ml_bf16()),
                "wq8": wq8,
                "wk8": wk8,
                "w1r": w1r,
                "w2r": w2r,
                "relx": relx,
            }
        )
    return in_maps


def assemble(cfg: Cfg, results):
    c = cfg
    out = np.zeros((c.B, c.L, c.E), np.float32)
    for core in range(c.n_cores):
        b = core // c.n_pair
        s = core % c.n_pair
        R0 = s * c.R
        oo = results[core]["out_own"]
        out[b, R0 : R0 + c.R] = oo.transpose(1, 0, 2).reshape(c.R, c.E)
    return out


# ---------------- public entry ----------------

_CACHE = {}


def _get_nc(cfg: Cfg):
    key = ("nc", cfg.L, cfg.D)
    if key not in _CACHE:
        import concourse.bacc as bacc

        nc = bacc.Bacc(
            "TRN2", target_bir_lowering=False, debug=False, num_devices=cfg.n_cores
        )
        build_decoder(nc, cfg)
        nc.compile()
        _CACHE[key] = nc
    return _CACHE[key]


def run(inputs, cfg: Cfg = FULL, trace: bool = False, **spmd_kwargs):
    from concourse.bass_utils import run_bass_kernel_spmd

    nc = _get_nc(cfg)
    in_maps = prep_inputs(cfg, inputs)
    res = run_bass_kernel_spmd(
        nc, in_maps, core_ids=list(range(cfg.n_cores)), trace=trace, **spmd_kwargs
    )
    out = assemble(cfg, res.results)
    return out, res


def kernel(**inputs):
    out, _ = run(inputs)
    return out.astype(np.float32)


# revision 15
# speedup vs baseline: 1.1834x; 1.1120x over previous
"""Trainium2 Bass kernel for nn_Decoder_11278584119887 (self-contained).

6-layer dense transformer decoder with head-averaged attention weights.
Sharding: 8 NeuronCores = 4 batch elements x 2 sequence halves; per-layer
bf16 AllGather (pairs), split into two halves so each half can overlap
compute.

Layout note: the m (key/value position) axis is kept in CORE-LOCAL order
[gatherA | gatherB] where gatherA = [core0 rows 0:RH | core1 rows 0:RH]
and RH = R/2.  This makes every DRAM source core-independent (SPMD safe).
rel columns and the layer-0 full input are host-permuted to match; the
softmax/sa math is invariant to a column permutation.

v2 vs baseline:
 - bulk DMAs moved to the Pool (SWDGE) queue + batched; transposes batched
   (one per chunk / l_tile); weights repacked host-side for contiguous
   reads.  (TimelineSim showed the SP dispatch queue saturated.)
 - Wq/Wk projections in fp8 e4m3 with DoubleRow (safe: w_avg only perturbs
   softmax(rel) weights by ~1e-3 multiplicatively).
 - scores head-pair matmuls interleaved for 64-row PE-tile concurrency.
 - rel bf16, preloaded into the softmax accumulator (saves the stage-2 add).
 - FFN emitted per sequence-half inside the scores sweep; LN3 + AllGather
   per half issued as soon as the half's rows are done.
"""

import sys as _sys

for _p in ("/opt/trn_rl_repo",):
    if _p not in _sys.path:
        _sys.path.insert(0, _p)

import math
from contextlib import ExitStack
from dataclasses import dataclass

import numpy as np

import concourse.bass as bass
import concourse.mybir as mybir
import concourse.tile as tile

F32 = mybir.dt.float32
BF16 = mybir.dt.bfloat16
FP8 = mybir.dt.float8e4
AF = mybir.ActivationFunctionType
OP = mybir.AluOpType
DR = mybir.MatmulPerfMode.DoubleRow

P = 128
EPS = 1e-5
DECAY = 16.0
CSCALE = 1.0


@dataclass
class Cfg:
    B: int = 4
    L: int = 2048
    E: int = 1024
    H: int = 16
    DH: int = 64
    F: int = 4096
    D: int = 6
    n_pair: int = 2

    @property
    def n_cores(self):
        return self.B * self.n_pair

    @property
    def R(self):
        return self.L // self.n_pair

    @property
    def RH(self):
        return self.R // 2

    @property
    def LT(self):
        return self.R // P

    @property
    def MT(self):
        return self.L // P

    @property
    def ET(self):
        return self.E // P

    @property
    def FT(self):
        return self.F // P


FULL = Cfg()
TINY = Cfg(B=1, L=512, E=512, H=8, DH=64, F=1024, D=2, n_pair=2)


def build_decoder(nc, cfg: Cfg, no_collective: bool = False, use_dr: bool = True):
    c = cfg
    ISD = 1.0 / math.sqrt(c.DH)
    LT, MT, ET, FT = c.LT, c.MT, c.ET, c.FT
    L, R, E, F, H, D, RH = c.L, c.R, c.E, c.F, c.H, c.D, c.RH
    SC = RH                      # hT chunk rows; must divide gather halves
    assert SC <= 512
    NSC = L // SC
    NOWN = R // SC               # own chunks (for Q)
    SCCH = min(1024, L)          # scores psum chunk (<=2 banks)
    NCH = L // SCCH
    SCJ = min(512, SCCH)
    NJ = SCCH // SCJ
    EH = min(512, E)
    NEH = E // EH
    GL = min(2, LT)              # l_tiles per sa group
    Lh = min(512, R)             # FFN row half
    NLh = R // Lh
    LhT = Lh // P
    FT2 = min(16, FT)            # W2 f-tiles per streamed chunk
    NFh = FT // FT2
    BST = min(512, E)
    NST = E // BST
    NPAIR = H // 2
    NB = max(1, min(4, 2 * RH // P, MT))  # m-tiles per sa hmt load
    DHH = c.DH
    MAGIC = 0x5F3759DF

    # ---- DRAM I/O ----
    xown_f32 = nc.dram_tensor("xown_f32", [P, LT, E], F32, kind="ExternalInput").ap()
    xown_bf = nc.dram_tensor("xown_bf", [R, E], BF16, kind="ExternalInput").ap()
    xfull_bf = nc.dram_tensor("xfull_bf", [L, E], BF16, kind="ExternalInput").ap()
    wq8_in = nc.dram_tensor("wq8", [D, P, ET, E], FP8, kind="ExternalInput").ap()
    wk8_in = nc.dram_tensor("wk8", [D, P, ET, E], FP8, kind="ExternalInput").ap()
    w1r_in = nc.dram_tensor("w1r", [D, P, FT, ET * P], BF16, kind="ExternalInput").ap()
    w2r_in = nc.dram_tensor("w2r", [D, P, ET, FT * P], BF16, kind="ExternalInput").ap()
    rel_in = nc.dram_tensor("relx", [LT, P, L], BF16, kind="ExternalInput").ap()
    out_own = nc.dram_tensor("out_own", [P, LT, E], F32, kind="ExternalOutput").ap()

    groups = [[2 * b, 2 * b + 1] for b in range(c.B)] if c.n_pair == 2 else None

    with tile.TileContext(nc) as tc, ExitStack() as ctx:
        singles = ctx.enter_context(tc.tile_pool(name="singles", bufs=1))
        dram = ctx.enter_context(tc.tile_pool(name="dram", bufs=1, space="DRAM"))
        ps_sc = ctx.enter_context(tc.tile_pool(name="ps_sc", bufs=2, space="PSUM"))
        ps_mm = ctx.enter_context(tc.tile_pool(name="ps_mm", bufs=2, space="PSUM"))
        ps_sa = ctx.enter_context(tc.tile_pool(name="ps_sa", bufs=2, space="PSUM"))
        smalls = ctx.enter_context(tc.tile_pool(name="smalls", bufs=2))

        # ---- persistent SBUF slabs ----
        h_own = singles.tile([P, LT, E], F32, name="h_own")
        KT = singles.tile([P, ET, L], BF16, name="KT")
        slabQ = singles.tile([P, ET * R], BF16, name="slabQ")
        QT = slabQ.rearrange("p (a b) -> p a b", a=ET)
        houtb = slabQ.rearrange("p (a b) -> p a b", a=LT)
        zTg = singles.tile([P, GL, MT, P], BF16, name="zTg")
        # sweep buffers; wq8/wk8 alias them (weights dead before the sweep)
        swslab = singles.tile([P, 4 * L], BF16, name="swslab")
        racc = [swslab[:, i * L : (i + 1) * L] for i in range(2)]
        e_pl = [swslab[:, (2 + i) * L : (3 + i) * L] for i in range(2)]
        wq8 = swslab[:, : ET * E // 2].bitcast(FP8).rearrange(
            "p (a b) -> p a b", a=ET
        )
        wk8 = swslab[:, ET * E // 2 : ET * E].bitcast(FP8).rearrange(
            "p (a b) -> p a b", a=ET
        )
        z_bf = singles.tile([P, L], BF16, name="z_bf")
        # hT chunks (bf16 + fp8); h2T overlays the bf16 chunk region
        slabD = singles.tile([P, 3 * ET * SC], BF16, name="slabD")
        hTb = [
            slabD[:, : ET * SC].rearrange("p (a b) -> p a b", a=ET),
            slabD[:, ET * SC : 2 * ET * SC].rearrange("p (a b) -> p a b", a=ET),
        ]
        h2T = slabD[:, : LT * ET * P].rearrange("p (t a b) -> p t a b", t=LT, a=ET)
        hT8 = [
            slabD[:, 2 * ET * SC : 2 * ET * SC + ET * SC // 2]
            .bitcast(FP8)
            .rearrange("p (a b) -> p a b", a=ET),
            slabD[:, 2 * ET * SC + ET * SC // 2 : 3 * ET * SC]
            .bitcast(FP8)
            .rearrange("p (a b) -> p a b", a=ET),
        ]
        # FFN slabs
        ff1T = singles.tile([P, FT, Lh], BF16, name="ff1T")
        ffT_sb = singles.tile([P, ET, Lh], BF16, name="ffT_sb")
        ff_rowT = singles.tile([P, ET * LhT, P], BF16, name="ff_rowT")
        ffv = ff_rowT.rearrange("p (a t) c -> p a t c", a=ET)
        # weight streams + hmt + h2st: manual double buffers in one slab
        wslab = singles.tile([P, 2 * ET * P + 2 * FT2 * P], BF16, name="wslab")
        w1b = [
            wslab[:, i * ET * P : (i + 1) * ET * P].rearrange(
                "p (a b) -> p a b", a=ET
            )
            for i in range(2)
        ]
        w2b = [
            wslab[
                :, 2 * ET * P + i * FT2 * P : 2 * ET * P + (i + 1) * FT2 * P
            ].rearrange("p (a b) -> p a b", a=FT2)
            for i in range(2)
        ]
        hmslab = singles.tile([P, 2 * NB * EH], BF16, name="hmslab")
        hmt = [
            hmslab[:, i * NB * EH : (i + 1) * NB * EH].rearrange(
                "p (a b) -> p a b", a=NB
            )
            for i in range(2)
        ]
        h2slab = singles.tile([P, 2 * E], BF16, name="h2slab")
        h2st = [h2slab[:, i * E : (i + 1) * E] for i in range(2)]
        # smalls packed into one f32 slab
        sm = singles.tile([P, 256], F32, name="sm")
        recip2 = sm[:, 0:LT]
        rs2 = sm[:, 16 : 16 + LT]
        c_eps = sm[:, 32:33]
        c_eps2 = sm[:, 33:34]
        rsh_s = [sm[:, 34 + i : 35 + i] for i in range(4)]
        mvt = sm[:, 40:42]
        sqt = sm[:, 42:43]
        st12 = sm[:, 48 : 48 + NST * 6].rearrange("p (a b) -> p a b", a=NST)
        st3 = sm[:, 64 : 64 + NST * 6].rearrange("p (a b) -> p a b", a=NST)
        mvb12 = sm[:, 96 : 96 + 2 * GL].rearrange("p (a b) -> p a b", a=GL)
        vg12 = sm[:, 104 : 104 + GL]
        rg12 = sm[:, 108 : 108 + GL]
        nt12 = sm[:, 112 : 112 + GL]
        mvh3 = sm[:, 192 : 192 + 2 * LhT].rearrange("p (a b) -> p a b", a=LhT)
        vh3 = sm[:, 208 : 208 + LhT]
        rh3 = sm[:, 216 : 216 + LhT]
        nth3 = sm[:, 224 : 224 + LhT]
        rsp_b = [
            sm[:, 128 + i * H * NCH : 128 + (i + 1) * H * NCH].rearrange(
                "p (a b) -> p a b", a=H
            )
            for i in range(2)
        ]
        nc.vector.memset(c_eps, float(EPS))
        nc.vector.memset(c_eps2, float(EPS * EPS))

        def rsqrt_dve(dst, src, tmp):
            """dst = 1/sqrt(src) on DVE (magic + 2 Newton steps)."""
            oi = dst.bitcast(mybir.dt.int32)
            nc.vector.tensor_scalar(
                out=oi, in0=src.bitcast(mybir.dt.int32), scalar1=1, scalar2=None,
                op0=OP.logical_shift_right,
            )
            nc.vector.tensor_scalar(
                out=oi, in0=oi, scalar1=MAGIC, scalar2=-1,
                op0=OP.subtract, op1=OP.mult,
            )
            for _ in range(2):
                nc.vector.tensor_tensor(out=tmp, in0=dst, in1=dst, op=OP.mult)
                nc.vector.tensor_tensor(out=tmp, in0=tmp, in1=src, op=OP.mult)
                nc.vector.tensor_scalar(
                    out=tmp, in0=tmp, scalar1=-0.5, scalar2=1.5,
                    op0=OP.mult, op1=OP.add,
                )
                nc.vector.tensor_tensor(out=dst, in0=dst, in1=tmp, op=OP.mult)

        # ---- DRAM exchange buffers (per parity, per half) ----
        hout_d = [
            [dram.tile([RH, E], BF16, name=f"hout{i}{j}") for j in range(2)]
            for i in range(2)
        ]
        hhalf_d = [
            [
                dram.tile([c.n_pair * RH, E], BF16, name=f"hh{i}{j}")
                for j in range(2)
            ]
            for i in range(2)
        ]

        def gcols(d, r0, nrows):
            """DRAM rows for local m-columns [r0, r0+nrows) of layer d input."""
            if d == 0:
                return xfull_bf[r0 : r0 + nrows]
            half = r0 // (c.n_pair * RH)
            off = r0 - half * c.n_pair * RH
            return hhalf_d[(d - 1) % 2][half][off : off + nrows]

        def own_rows(d, r0, nrows):
            """Own rows (local l index) of layer d input."""
            if d == 0:
                return xown_bf[r0 : r0 + nrows]
            half = r0 // RH
            off = r0 - half * RH
            return hout_d[(d - 1) % 2][half][off : off + nrows]

        nc.gpsimd.dma_start(out=h_own[:], in_=xown_f32[:])

        def proj(idx, buf, src, wt, dst):
            """One hT chunk: transpose src rows -> fp8 -> project with wt
            into dst columns [idx*SC, (idx+1)*SC)."""
            nc.sync.dma_start_transpose(hTb[buf][:], src)
            nc.scalar.activation(
                out=hT8[buf][:], in_=hTb[buf][:], func=AF.Copy, scale=1.0
            )
            for ept in range(ET):
                ps = ps_mm.tile([P, 512], F32, tag="mm", name="ps_p")
                if use_dr and ET % 2 == 0:
                    for jp in range(ET // 2):
                        nc.tensor.matmul(
                            ps[:, :SC],
                            wt[:, 2 * jp : 2 * jp + 2, ept * P : (ept + 1) * P],
                            hT8[buf][:, 2 * jp : 2 * jp + 2, :],
                            start=(jp == 0),
                            stop=(jp == ET // 2 - 1),
                            perf_mode=DR,
                        )
                else:
                    for et in range(ET):
                        nc.tensor.matmul(
                            ps[:, :SC],
                            wt[:, et, ept * P : (ept + 1) * P],
                            hT8[buf][:, et, :],
                            start=(et == 0),
                            stop=(et == ET - 1),
                        )
                nc.vector.tensor_copy(
                    out=dst[:, ept, idx * SC : (idx + 1) * SC], in_=ps[:, :SC]
                )

        def ffn_half(d, lh):
            """FFN + LN3 for l_tiles [lh*LhT, (lh+1)*LhT); store + collective
            for any fully-finished row half."""
            last = d == D - 1
            with nc.named_scope(f"d{d}ffn{lh}"):
                for ft in range(FT):
                    wb = w1b[ft % 2]
                    nc.sync.dma_start(out=wb[:], in_=w1r_in[d, :, ft])
                    ps = ps_mm.tile([P, 512], F32, tag="mm", name="ps_f1")
                    for et in range(ET):
                        nc.tensor.matmul(
                            ps[:, :Lh],
                            wb[:, et, :],
                            h2T[:, lh * LhT : (lh + 1) * LhT, et, :],
                            start=(et == 0),
                            stop=(et == ET - 1),
                        )
                    nc.vector.tensor_scalar(
                        out=ff1T[:, ft, :], in0=ps[:, :Lh], scalar1=0.0,
                        scalar2=None, op0=OP.max,
                    )
                for ept in range(ET):
                    ps2 = ps_mm.tile([P, 512], F32, tag="mm", name="ps_f2")
                    for fh in range(NFh):
                        wb2 = w2b[fh % 2]
                        nc.sync.dma_start(
                            out=wb2[:],
                            in_=w2r_in[d, :, ept, fh * FT2 * P : (fh + 1) * FT2 * P],
                        )
                        for f2 in range(FT2):
                            nc.tensor.matmul(
                                ps2[:, :Lh],
                                wb2[:, f2, :],
                                ff1T[:, fh * FT2 + f2, :],
                                start=(fh == 0 and f2 == 0),
                                stop=(fh == NFh - 1 and f2 == FT2 - 1),
                            )
                    nc.vector.tensor_copy(out=ffT_sb[:, ept, :], in_=ps2[:, :Lh])
                nc.sync.dma_start_transpose(ff_rowT[:], ffT_sb[:])
                # res2 + LN3 per tile of this half
                for tl in range(LhT):
                    tt = lh * LhT + tl
                    nc.vector.scalar_tensor_tensor(
                        out=h_own[:, tt, :],
                        in0=ffv[:, :, tl, :],
                        scalar=1.0,
                        in1=h_own[:, tt, :],
                        op0=OP.mult,
                        op1=OP.add,
                    )
                    for i in range(NST):
                        nc.vector.bn_stats(
                            out=st3[:, i, :],
                            in_=h_own[:, tt, i * BST : (i + 1) * BST],
                        )
                    nc.vector.bn_aggr(out=mvh3[:, tl, :], in_=st3[:])
                nc.vector.tensor_scalar(
                    out=vh3, in0=mvh3[:, :, 1], scalar1=float(EPS), scalar2=None,
                    op0=OP.add,
                )
                rsqrt_dve(rh3, vh3, nth3)
                for tl in range(LhT):
                    tt = lh * LhT + tl
                    nc.vector.tensor_scalar(
                        out=h_own[:, tt, :], in0=h_own[:, tt, :],
                        scalar1=mvh3[:, tl, 0:1], scalar2=rh3[:, tl : tl + 1],
                        op0=OP.subtract, op1=OP.mult,
                    )
                    if not last:
                        nc.vector.tensor_copy(
                            out=houtb[:, tt, :], in_=h_own[:, tt, :]
                        )
                if last:
                    return
                # store + collective for finished halves
                assert RH >= P
                done_rows = (lh + 1) * LhT * P
                for hf in range(2):
                    lo, hi = hf * RH, (hf + 1) * RH
                    if done_rows < hi or lh * LhT * P >= hi:
                        continue
                    dst = hout_d[d % 2][hf]
                    tpr = RH // P
                    nc.gpsimd.dma_start(
                        out=dst.rearrange("(t p) e -> p t e", p=P),
                        in_=houtb[:, hf * tpr : (hf + 1) * tpr, :],
                    )
                    if no_collective:
                        hh = hhalf_d[d % 2][hf]
                        nc.sync.dma_start(out=hh[:RH], in_=dst[:])
                        nc.sync.dma_start(out=hh[RH:], in_=dst[:])
                    else:
                        nc.gpsimd.collective_compute(
                            "AllGather",
                            OP.bypass,
                            replica_groups=groups,
                            ins=[dst.opt()],
                            outs=[hhalf_d[d % 2][hf].opt()],
                        )

        for d in range(D):
            last = d == D - 1
            with nc.named_scope(f"d{d}proj"):
                nc.gpsimd.dma_start(out=wq8[:], in_=wq8_in[d])
                nc.gpsimd.dma_start(out=wk8[:], in_=wk8_in[d])
                # Q chunks first (own rows, no collective dependency)
                for qi in range(NOWN):
                    proj(qi, qi % 2, own_rows(d, qi * SC, SC), wq8, QT)
                for ci in range(NSC):
                    proj(ci, ci % 2, gcols(d, ci * SC, SC), wk8, KT)

            # ---- scores sweep ----
            for t in range(LT):
                with nc.named_scope(f"d{d}sw{t}"):
                    rb = racc[t % 2]
                    nc.gpsimd.dma_start(out=rb[:], in_=rel_in[t])
                    rsp = rsp_b[t % 2]
                    for q in range(NPAIR):
                        h0, h1 = 2 * q, 2 * q + 1
                        for ch in range(NCH):
                            pA = ps_sc.tile([P, SCCH], F32, tag="sc", name="psA")
                            pB = ps_sc.tile([P, SCCH], F32, tag="sc", name="psB")
                            for j in range(NJ):
                                m0 = ch * SCCH + j * SCJ
                                for hh, pp in ((h0, pA), (h1, pB)):
                                    pof = DHH * (hh % 2)
                                    nc.tensor.matmul(
                                        pp[:, j * SCJ : (j + 1) * SCJ],
                                        QT[pof : pof + DHH, hh // 2,
                                           t * P : (t + 1) * P],
                                        KT[pof : pof + DHH, hh // 2,
                                           m0 : m0 + SCJ],
                                        start=True,
                                        stop=True,
                                    )
                            for hh, pp in ((h0, pA), (h1, pB)):
                                nc.scalar.activation(
                                    out=e_pl[hh % 2][:, ch * SCCH : (ch + 1) * SCCH],
                                    in_=pp[:],
                                    func=AF.Exp,
                                    scale=ISD,
                                    accum_out=rsp[:, hh, ch : ch + 1],
                                )
                        for hh in (h0, h1):
                            eb = e_pl[hh % 2]
                            rsh = rsh_s[hh % 4]
                            if NCH > 1:
                                nc.vector.tensor_reduce(
                                    out=rsh, in_=rsp[:, hh, :],
                                    axis=mybir.AxisListType.X, op=OP.add,
                                )
                                nc.vector.reciprocal(out=rsh, in_=rsh)
                            else:
                                nc.vector.reciprocal(out=rsh, in_=rsp[:, hh, :])
                            nc.vector.tensor_scalar(
                                out=eb[:], in0=eb[:], scalar1=rsh,
                                scalar2=float(CSCALE / H),
                                op0=OP.mult, op1=OP.mult,
                            )
                            nc.vector.tensor_tensor(
                                out=rb[:], in0=eb[:], in1=rb[:], op=OP.add
                            )
                    nc.scalar.activation(
                        out=z_bf[:], in_=rb[:], func=AF.Exp, scale=1.0,
                        accum_out=rs2[:, t : t + 1],
                    )
                    nc.sync.dma_start_transpose(zTg[:, t % GL], z_bf[:])

                if t % GL == GL - 1:
                    g = t // GL
                    t0 = g * GL
                    with nc.named_scope(f"d{d}sa{g}"):
                        nc.vector.reciprocal(
                            out=recip2[:, t0 : t0 + GL], in_=rs2[:, t0 : t0 + GL]
                        )
                        for eh in range(NEH):
                            pss = [
                                ps_sa.tile([P, EH], F32, tag="sa", name="ps_sa")
                                for _ in range(GL)
                            ]
                            for mi, ms in enumerate(range(0, MT, NB)):
                                hb = hmt[mi % 2]
                                nc.sync.dma_start(
                                    out=hb[:],
                                    in_=gcols(d, ms * P, NB * P).rearrange(
                                        "(a p) e -> p a e", p=P
                                    )[:, :, eh * EH : (eh + 1) * EH],
                                )
                                for tg in range(GL):
                                    for k in range(NB):
                                        nc.tensor.matmul(
                                            pss[tg],
                                            zTg[:, tg, ms + k, :],
                                            hb[:, k, :],
                                            start=(mi == 0 and k == 0),
                                            stop=(ms + k == MT - 1),
                                        )
                            for tg in range(GL):
                                nc.vector.scalar_tensor_tensor(
                                    out=h_own[:, t0 + tg, eh * EH : (eh + 1) * EH],
                                    in0=pss[tg],
                                    scalar=recip2[:, t0 + tg : t0 + tg + 1],
                                    in1=h_own[:, t0 + tg, eh * EH : (eh + 1) * EH],
                                    op0=OP.mult,
                                    op1=OP.add,
                                )
                        # LN1+LN2 fused, h2 in place (f32) + bf16 copy + hT
                        for tg in range(GL):
                            tt = t0 + tg
                            for i in range(NST):
                                nc.vector.bn_stats(
                                    out=st12[:, i, :],
                                    in_=h_own[:, tt, i * BST : (i + 1) * BST],
                                )
                            nc.vector.bn_aggr(out=mvb12[:, tg, :], in_=st12[:])
                        # u = v*(1+eps)+eps^2 for the group; rstd via DVE rsqrt
                        nc.vector.tensor_scalar(
                            out=vg12, in0=mvb12[:, :, 1], scalar1=float(1.0 + EPS),
                            scalar2=float(EPS * EPS), op0=OP.mult, op1=OP.add,
                        )
                        rsqrt_dve(rg12, vg12, nt12)
                        for tg in range(GL):
                            tt = t0 + tg
                            nc.vector.tensor_scalar(
                                out=h_own[:, tt, :], in0=h_own[:, tt, :],
                                scalar1=mvb12[:, tg, 0:1],
                                scalar2=rg12[:, tg : tg + 1],
                                op0=OP.subtract, op1=OP.mult,
                            )
                            hs = h2st[tg % 2]
                            nc.vector.tensor_copy(out=hs[:], in_=h_own[:, tt, :])
                            nc.sync.dma_start_transpose(h2T[:, tt], hs[:])
                    # FFN for any completed row half
                    for lh in range(NLh):
                        if (lh + 1) * Lh == (t + 1) * P:
                            ffn_half(d, lh)

        nc.gpsimd.dma_start(out=out_own[:], in_=h_own[:])


# ---------------- host-side helpers ----------------


def make_rel(L):
    pos = np.arange(L)
    return np.exp(-np.abs(pos[:, None] - pos[None, :]).astype(np.float32) / DECAY)


def perm_cols(c: Cfg):
    """Core-local m order: [gatherA | gatherB] halves."""
    L, R, RH = c.L, c.R, c.RH
    return np.concatenate(
        [
            np.arange(0, RH),
            np.arange(R, R + RH),
            np.arange(RH, R),
            np.arange(R + RH, L),
        ]
    )


def ml_bf16():
    import ml_dtypes

    return ml_dtypes.bfloat16


def ml_fp8():
    import ml_dtypes

    return ml_dtypes.float8_e4m3


def prep_inputs(cfg: Cfg, inputs):
    c = cfg
    x = np.asarray(inputs["x"], np.float32)
    Wq = np.asarray(inputs["Wq"], np.float32)
    Wk = np.asarray(inputs["Wk"], np.float32)
    W1 = np.asarray(inputs["W1"], np.float32)
    W2 = np.asarray(inputs["W2"], np.float32)
    rel = make_rel(c.L)
    pc = perm_cols(c)

    def to_lhsT(w):  # [D, out, in] -> [D, P, in_tiles, out]
        D_, O_, I_ = w.shape
        wT = np.ascontiguousarray(np.transpose(w, (0, 2, 1)))
        return np.ascontiguousarray(
            wT.reshape(D_, I_ // P, P, O_).transpose(0, 2, 1, 3)
        )

    wq8 = to_lhsT(Wq).astype(ml_fp8())
    wk8 = to_lhsT(Wk).astype(ml_fp8())
    w1T = to_lhsT(W1)  # [D, P, ET, F]
    D_, _, ET_, F_ = w1T.shape
    FT_ = F_ // P
    w1r = np.ascontiguousarray(
        w1T.reshape(D_, P, ET_, FT_, P).transpose(0, 1, 3, 2, 4)
    ).reshape(D_, P, FT_, ET_ * P).astype(ml_bf16())
    w2T = to_lhsT(W2)  # [D, P, FT, E]
    _, _, FT2_, E_ = w2T.shape
    ET2_ = E_ // P
    w2r = np.ascontiguousarray(
        w2T.reshape(D_, P, FT2_, ET2_, P).transpose(0, 1, 3, 2, 4)
    ).reshape(D_, P, ET2_, FT2_ * P).astype(ml_bf16())

    in_maps = []
    for core in range(c.n_cores):
        b = core // c.n_pair
        s = core % c.n_pair
        R0 = s * c.R
        xrows = x[b, R0 : R0 + c.R]
        xown_f32 = np.ascontiguousarray(xrows.reshape(c.LT, P, c.E).transpose(1, 0, 2))
        relx = np.ascontiguousarray(
            rel[R0 : R0 + c.R][:, pc].reshape(c.LT, P, c.L)
        ).astype(ml_bf16())
        in_maps.append(
            {
                "xown_f32": xown_f32,
                "xown_bf": xrows.astype(ml_bf16()),
                "xfull_bf": np.ascontiguousarray(x[b][pc]).astype(ml_bf16()),
                "wq8": wq8,
                "wk8": wk8,
                "w1r": w1r,
                "w2r": w2r,
                "relx": relx,
            }
        )
    return in_maps


def assemble(cfg: Cfg, results):
    c = cfg
    out = np.zeros((c.B, c.L, c.E), np.float32)
    for core in range(c.n_cores):
        b = core // c.n_pair
        s = core % c.n_pair
        R0 = s * c.R
        oo = results[core]["out_own"]
        out[b, R0 : R0 + c.R] = oo.transpose(1, 0, 2).reshape(c.R, c.E)
    return out


# ---------------- public entry ----------------

_CACHE = {}


def _get_nc(cfg: Cfg):
    key = ("nc", cfg.L, cfg.D)
    if key not in _CACHE:
        import concourse.bacc as bacc

        nc = bacc.Bacc(
            "TRN2", target_bir_lowering=False, debug=False, num_devices=cfg.n_cores
        )
        build_decoder(nc, cfg)
        nc.compile()
        _CACHE[key] = nc
    return _CACHE[key]


def run(inputs, cfg: Cfg = FULL, trace: bool = False, **spmd_kwargs):
    from concourse.bass_utils import run_bass_kernel_spmd

    nc = _get_nc(cfg)
    in_maps = prep_inputs(cfg, inputs)
    res = run_bass_kernel_spmd(
        nc, in_maps, core_ids=list(range(cfg.n_cores)), trace=trace, **spmd_kwargs
    )
    out = assemble(cfg, res.results)
    return out, res


def kernel(**inputs):
    out, _ = run(inputs)
    return out.astype(np.float32)
